# revision 14
# baseline (speedup 1.0000x reference)
"""Trainium2 Bass kernel for a dense transformer block (B=4, T=2048, D=1024, H=16).

Sharding: 8 cores = 4 batches x 2 query-parity groups. Each core computes
LN1+QKV (K/V for the full sequence, all heads) for its batch, then causal
attention + proj + FFN for its half of the query blocks only. Query blocks are
assigned in a zigzag so causal attention work is balanced across the two cores
of a batch:
  parity 0 -> blocks [15,12,11,8,7,4,3,0]   (causal lengths 16,13,12,9,8,5,4,1)
  parity 1 -> blocks [14,13,10,9,6,5,2,1]   (causal lengths 15,14,11,10,7,6,3,2)
Slot j on every core processes NKMAX[j] = [16,14,12,10,8,6,4,2] key blocks, so
the SPMD program is identical on all 8 cores; the causal mask for the last two
key blocks of each slot is an input (differs by parity). The host permutes each
core's query tokens into slot order and inverse-permutes the output.

Matmul operands are bf16 (fp32 PE matmul is 4x slower); accumulation, softmax
denominators, layernorm stats and residuals are fp32. Softmax skips the
max-subtraction (scores are ~N(0,1) after the 1/8 scale; exp is exact softmax
mathematically and safe in fp32/bf16 range here), which lets the denominator
come from an appended ones-column in V during the PV matmul.

The PE is in-order, so emission order is the schedule. The body is one
software-pipelined stream: LN1 tiles feed transposes feed QKV matmuls at
token-block granularity; attention stages (one head x one slot) interleave
score matmuls, exp (ACT), mask (Pool/GpSimd), and PV so the PE never idles on
the exp drain; K-pair-1 production, per-slot proj, LN2 and XT2 transposes are
injected between stages as PE filler. FFN weights prefetch during the
attention tail.
"""

import sys

try:
    import concourse  # noqa: F401
except ImportError:
    sys.path.insert(0, "/opt/trn_rl_repo")

import numpy as np
import ml_dtypes

import concourse.bass as bass
import concourse.bacc as bacc
import concourse.tile as tile
from concourse import mybir
from concourse.bass_utils import run_bass_kernel_spmd

F32 = mybir.dt.float32
BF16 = mybir.dt.bfloat16
AF = mybir.ActivationFunctionType
ALU = mybir.AluOpType
PSUM = bass.MemorySpace.PSUM

B, T, D = 4, 2048, 1024
H, HD = 16, 64
HID = 4 * D
EPS = 1e-5
N_CORES = 8
NB = T // 128  # 16 token blocks per batch
NQ = 8  # query slots per core
TQ = NQ * 128  # query tokens per core

QB = [
    [15, 12, 11, 8, 7, 4, 3, 0],
    [14, 13, 10, 9, 6, 5, 2, 1],
]
NKMAX = [16, 14, 12, 10, 8, 6, 4, 2]


def _perm(p):
    idx = []
    for qb in QB[p]:
        idx.extend(range(qb * 128, (qb + 1) * 128))
    return np.array(idx, dtype=np.int64)


def _masks(p):
    """[NQ, 128, 256] multiplicative mask for the last two key blocks per slot."""
    m = np.zeros((NQ, 128, 256), dtype=np.float32)
    tri = np.triu(np.ones((128, 128), np.float32))  # valid where k_local <= q_local
    for j in range(NQ):
        nk = QB[p][j] + 1
        nkm = NKMAX[j]
        for c in range(2):
            ki = nkm - 2 + c
            if ki < nk - 1:
                m[j, :, c * 128:(c + 1) * 128] = 1.0
            elif ki == nk - 1:
                m[j, :, c * 128:(c + 1) * 128] = tri
    return m


# ---------------------------------------------------------------------------
# Device program
# ---------------------------------------------------------------------------

def _ln_stats(nc, pool, x_ap, eps_t):
    """LN stats of a [128, D] fp32 tile -> (rstd, nmr) [128,1] tiles."""
    stats = pool.tile([128, 2, nc.vector.BN_STATS_DIM], F32, tag="ln_stats")
    mv = pool.tile([128, nc.vector.BN_AGGR_DIM], F32, tag="ln_mv")
    nmr = pool.tile([128, 1], F32, tag="ln_nmr")
    xg = x_ap.rearrange("p (s d) -> p s d", s=2)
    for s in range(2):
        nc.vector.bn_stats(out=stats[:, s, :], in_=xg[:, s, :])
    nc.vector.bn_aggr(out=mv[:], in_=stats[:])
    mean = mv[:, 0:1]
    rstd = mv[:, 1:2]
    nc.scalar.activation(out=rstd, in_=rstd, func=AF.Sqrt, bias=eps_t[:], scale=1.0)
    nc.vector.reciprocal(out=rstd, in_=rstd)
    nc.vector.tensor_scalar(
        out=nmr[:], in0=mean, scalar1=rstd, scalar2=-1.0,
        op0=ALU.mult, op1=ALU.mult,
    )
    return rstd, nmr


def build_program(niter=None, stop_after=None):
    nc = bacc.Bacc("TRN2", target_bir_lowering=False, debug=False,
                   num_devices=N_CORES)

    xf_d = nc.dram_tensor("x_full", [T, D], F32, kind="ExternalInput")
    xq_d = nc.dram_tensor("x_q", [TQ, D], F32, kind="ExternalInput")
    mask_d = nc.dram_tensor("mask", [128, NQ, 256], BF16, kind="ExternalInput")
    wqkv_d = nc.dram_tensor("wqkv", [128, 8, 3 * D], BF16, kind="ExternalInput")
    wproj_d = nc.dram_tensor("wproj", [128, 8, D], BF16, kind="ExternalInput")
    wff1_d = nc.dram_tensor("wff1", [128, 8, HID], BF16, kind="ExternalInput")
    wff2_d = nc.dram_tensor("wff2", [128, 32, D], BF16, kind="ExternalInput")
    bqk_d = nc.dram_tensor("bqk", [128, 16], F32, kind="ExternalInput")
    bv_d = nc.dram_tensor("bv", [D], F32, kind="ExternalInput")
    bproj_d = nc.dram_tensor("bproj", [D], F32, kind="ExternalInput")
    bff1_d = nc.dram_tensor("bff1", [128, 32], F32, kind="ExternalInput")
    bff2_d = nc.dram_tensor("bff2", [D], F32, kind="ExternalInput")
    g1_d = nc.dram_tensor("g1", [128, 8], F32, kind="ExternalInput")
    b1_d = nc.dram_tensor("b1", [128, 8], F32, kind="ExternalInput")
    g2_d = nc.dram_tensor("g2", [128, 8], F32, kind="ExternalInput")
    b2_d = nc.dram_tensor("b2", [128, 8], F32, kind="ExternalInput")
    y_d = nc.dram_tensor("y", [TQ, D], F32, kind="ExternalOutput")

    def bcast_row(dram_handle, n):
        ap = dram_handle.ap()
        return bass.AP(tensor=ap.tensor, offset=ap.offset, ap=[[0, 128], [1, n]])

    dram = dict(xf=xf_d, xq=xq_d, wqkv=wqkv_d, wproj=wproj_d,
                wff1=wff1_d, wff2=wff2_d, y=y_d)

    with tile.TileContext(nc) as tc:
        with tc.tile_pool(name="singles", bufs=1) as singles:
            ident = singles.tile([128, 128], BF16)
            from concourse.masks import make_identity
            make_identity(nc, ident[:])
            nc._ident = ident

            st = {}
            st["eps"] = singles.tile([128, 1], F32, name="eps", tag="eps")
            nc.vector.memset(st["eps"][:], EPS)
            for nm, dt, shape, src in (
                ("g1", F32, [128, 8], g1_d), ("b1", F32, [128, 8], b1_d),
                ("g2", F32, [128, 8], g2_d), ("b2", F32, [128, 8], b2_d),
                ("bqk", F32, [128, 16], bqk_d), ("bff1", F32, [128, 32], bff1_d),
                ("mask", BF16, [128, NQ, 256], mask_d),
            ):
                st[nm] = singles.tile(shape, dt, name=nm, tag=nm)
                nc.sync.dma_start(st[nm][:], src.ap())
            for nm, src in (("vb", bv_d), ("pb", bproj_d), ("fb2", bff2_d)):
                st[nm] = singles.tile([128, D], F32, name=nm, tag=nm)
                nc.gpsimd.dma_start(st[nm][:], bcast_row(src, D))

            if niter is None:
                _block_body(tc, nc, dram, st, stop_after)
            else:
                with tc.For_i(0, niter, 1):
                    _block_body(tc, nc, dram, st, stop_after)

    nc.compile()
    return nc


def _block_body(tc, nc, dram, st, stop_after=None):
    xf_d, xq_d, y_d = dram["xf"], dram["xq"], dram["y"]
    eps_t = st["eps"]
    x2_d = nc.dram_tensor("x2_scratch", [TQ, D], F32, kind="Internal")

    # --- persistent SBUF (right side, LIFO release order) -----------------
    # XTMP serves as XTQ during the prefix and is recycled as XT2 during
    # attention (XTQ's last read precedes XT2's first write; tile WAR deps
    # order them).  w1p sits at the bottom of the left stack so it can
    # outlive the attention pools into ff1.
    w1p = tc.alloc_tile_pool(name="w1", bufs=4)
    xtmp_p = tc.alloc_tile_pool(name="xtmp", bufs=1, side="right")
    XTQ = xtmp_p.tile([128, 8, TQ], BF16, name="XTMP", tag="xtmp")
    XT2 = XTQ
    qkv_p = tc.alloc_tile_pool(name="qkv", bufs=1, side="right")
    QT = qkv_p.tile([128, 8, TQ], BF16, name="QT", tag="qt")
    KT = qkv_p.tile([128, 8, T], BF16, name="KT", tag="kt")
    V = qkv_p.tile([128, NB, H, HD + 1], BF16, name="V", tag="v")
    for t in range(NB):
        nc.gpsimd.memset(V[:, t, :, HD:HD + 1], 1.0)

    xtf_p = tc.alloc_tile_pool(name="xtf", bufs=1, side="right")
    XTF = xtf_p.tile([128, 8, T], BF16, name="XTF", tag="xtf")

    # --- PSUM pools: sc 3x[128,1024]f32 (6 banks) + u 2x2KB (2 banks) -----
    sc_ps = tc.alloc_tile_pool(name="sc_ps", bufs=2, space=PSUM)
    u_ps = tc.alloc_tile_pool(name="u_ps", bufs=4, space=PSUM)

    def sc_tile():
        return sc_ps.tile([128, 1024], F32, tag="sc", name="ps")

    def u_tile(dt=F32):
        if dt == F32:
            return u_ps.tile([128, 512], dt, tag="u", name="ups")
        return u_ps.tile([128, 512], dt, tag="u", name="ups",
                         padded_shape=[128, 1024])

    # --- prefix pools (right side, above xtf) -----------------------------
    wqk_p = tc.alloc_tile_pool(name="wqk", bufs=3, side="right")
    wv_p = tc.alloc_tile_pool(name="wv", bufs=2, side="right")
    p1 = tc.alloc_tile_pool(name="p1", bufs=5, side="right")
    p1s = tc.alloc_tile_pool(name="p1s", bufs=8, side="right")
    xln_p = tc.alloc_tile_pool(name="xln", bufs=4, side="right")

    # ---- LN1 pipeline pieces --------------------------------------------
    def ln1_tile(x_d, row0):
        """Load [128, D] from x_d at row0, LN -> bf16 tile (no gain yet)."""
        xt = p1.tile([128, D], F32, tag="xt", name="xt")
        nc.sync.dma_start(xt[:], x_d.ap()[row0:row0 + 128, :])
        ln = xln_p.tile([128, D], BF16, tag="ln", name="ln")
        rstd, nmr = _ln_stats(nc, p1s, xt[:], eps_t)
        nc.scalar.activation(out=ln[:], in_=xt[:], func=AF.Identity,
                             bias=nmr[:], scale=rstd)
        return ln

    def tp_group(tiles4, dst3, col0):
        """Transpose 4 LN tiles' dim-chunks into dst3[:, dc, col0:col0+512].

        g1/b1 fused into the psum->sbuf copy (DVE even dc / ACT odd dc)."""
        for dc in range(8):
            pt = u_tile(BF16)
            for u in range(4):
                nc.tensor.transpose(
                    pt[:, u * 128:(u + 1) * 128],
                    tiles4[u][:, dc * 128:(dc + 1) * 128], nc._ident[:])
            dst = dst3[:, dc, col0:col0 + 512]
            if dc % 2 == 1:
                nc.scalar.activation(
                    out=dst, in_=pt[:], func=AF.Identity,
                    bias=st["b1"][:, dc:dc + 1], scale=st["g1"][:, dc:dc + 1])
            else:
                nc.vector.tensor_scalar(
                    out=dst, in0=pt[:],
                    scalar1=st["g1"][:, dc:dc + 1], scalar2=st["b1"][:, dc:dc + 1],
                    op0=ALU.mult, op1=ALU.add)

    def make_qk_half(cc, pair, h2, w):
        """One [128,512] half of a QT (cc<8) / KT (cc>=8) chunk via u_ps."""
        is_q = cc < 8
        XT = XTQ if is_q else XTF
        OUT = QT if is_q else KT
        occ = cc if is_q else cc - 8
        ps = u_tile()
        for dc in range(8):
            nc.tensor.matmul(
                ps[:], w[:, dc, :],
                XT[:, dc, (pair * 2 + h2) * 512:(pair * 2 + h2 + 1) * 512],
                start=(dc == 0), stop=(dc == 7),
            )
        nc.vector.tensor_scalar_add(
            out=OUT[:, occ, (pair * 2 + h2) * 512:(pair * 2 + h2 + 1) * 512],
            in0=ps[:], scalar1=st["bqk"][:, cc:cc + 1],
        )

    def load_wqk(cc):
        w = wqk_p.tile([128, 8, 128], BF16, tag="wqk", name="w")
        nc.gpsimd.dma_start(
            w[:], dram["wqkv"].ap()[:, :, cc * 128:(cc + 1) * 128])
        return w

    def v_block(t, vh, wv):
        """V for token block t, head-half vh (8 heads x 64) via u_ps."""
        ps = u_tile()
        for dc in range(8):
            nc.tensor.matmul(
                ps[:], XTF[:, dc, t * 128:(t + 1) * 128],
                wv[:, dc, :], start=(dc == 0), stop=(dc == 7),
            )
        nc.vector.tensor_add(
            out=V[:, t, vh * 8:(vh + 1) * 8, 0:HD],
            in0=ps[:].rearrange("p (h d) -> p h d", h=8),
            in1=st["vb"][:, vh * 512:(vh + 1) * 512]
                .rearrange("p (h d) -> p h d", h=8),
        )

    # =====================================================================
    # PREFIX: pipelined LN1 -> transposes -> K0 / Q / V.  Emission order is
    # the PE schedule; LN for later groups overlaps earlier PE matmuls.
    # =====================================================================
    wv0 = wv_p.tile([128, 8, 512], BF16, tag="wv", name="wv0")
    nc.gpsimd.dma_start(wv0[:], dram["wqkv"].ap()[:, :, 2 * D:2 * D + 512])
    wv1 = wv_p.tile([128, 8, 512], BF16, tag="wv", name="wv1")
    nc.gpsimd.dma_start(wv1[:], dram["wqkv"].ap()[:, :, 2 * D + 512:3 * D])

    # xf group 0 (blocks 0..3) -> XTF cols 0..512
    tiles = [ln1_tile(xf_d, u * 128) for u in range(4)]
    tp_group(tiles, XTF, 0)

    # K pair-0 lower halves (tokens 0..512) on PE; xf group 1 + xq LN
    # tiles stream on ACT/DVE meanwhile.  Weight chunks are re-loaded per
    # half-pass (cheap: gpsimd-issued DMA) to keep the wqk pool at 3 bufs.
    ln_feed = [lambda u=u: ln1_tile(xf_d, (4 + u) * 128) for u in range(4)]
    ln_feed += [lambda u=u: ln1_tile(xq_d, u * 128) for u in range(8)]
    ln_out = []

    def feed(n):
        for _ in range(n):
            if ln_feed:
                ln_out.append(ln_feed.pop(0)())

    wk = [load_wqk(8), load_wqk(9)]
    for i, cc in enumerate(range(8, 16)):
        if cc + 2 < 16:
            wk.append(load_wqk(cc + 2))
        feed(1)
        make_qk_half(cc, 0, 0, wk[i])
    tp_group(ln_out[0:4], XTF, 512)  # xf group 1

    # K pair-0 upper halves (tokens 512..1024); remaining xq LN streams
    wk = [load_wqk(8), load_wqk(9)]
    for i, cc in enumerate(range(8, 16)):
        if cc + 2 < 16:
            wk.append(load_wqk(cc + 2))
        feed(1)
        make_qk_half(cc, 0, 1, wk[i])
    tp_group(ln_out[4:8], XTQ, 0)
    tp_group(ln_out[8:12], XTQ, 512)

    # Q (all chunks, both halves); xf group 2 LN meanwhile
    ln_feed = [lambda u=u: ln1_tile(xf_d, (8 + u) * 128) for u in range(8)]
    ln_out = []
    wk = [load_wqk(0), load_wqk(1)]
    for cc in range(8):
        if cc + 2 < 8:
            wk.append(load_wqk(cc + 2))
        feed(1)
        make_qk_half(cc, 0, 0, wk[cc])
        make_qk_half(cc, 0, 1, wk[cc])

    # V blocks 0..7; xf group 3 LN meanwhile
    for t in range(8):
        feed(1)
        v_block(t, 0, wv0)
        v_block(t, 1, wv1)
    # xf groups 2,3 transposes -> XTF cols 1024..2048
    tp_group(ln_out[0:4], XTF, 1024)
    tp_group(ln_out[4:8], XTF, 1536)
    # V blocks 8..15
    for t in range(8, 16):
        v_block(t, 0, wv0)
        v_block(t, 1, wv1)

    xln_p.release()
    p1s.release()
    p1.release()
    wv_p.release()
    if stop_after == "prefix":
        wqk_p.release()
        xtf_p.release()
        qkv_p.release()
        xtmp_p.release()
        u_ps.release()
        sc_ps.release()
        w1p.release()
        return

    # =====================================================================
    # ATTENTION: stages = (slot j desc, head h asc). Injected between
    # stages: K pair 1 halves, per-slot proj/LN2/XT2, weight prefetch.
    # =====================================================================
    atp = tc.alloc_tile_pool(name="at", bufs=4)
    epp = tc.alloc_tile_pool(name="ep", bufs=4)
    attt_pool = tc.alloc_tile_pool(name="atttsl", bufs=2)
    att_pool = tc.alloc_tile_pool(name="attsl", bufs=2)
    x2t_pool = tc.alloc_tile_pool(name="x2tp", bufs=1)
    xqp = tc.alloc_tile_pool(name="xqp", bufs=1)
    ln2p = tc.alloc_tile_pool(name="ln2p", bufs=2)
    wpp = tc.alloc_tile_pool(name="wproj", bufs=1)

    PW = wpp.tile([128, 8, D], BF16, name="PW", tag="pw")
    nc.gpsimd.dma_start(PW[:], dram["wproj"].ap())

    # ---- injected tasks (popped 1-2 per stage, FIFO) --------------------
    inj = []
    k1_w = {}

    def mk_k1_load(cc):
        def run():
            k1_w[cc] = load_wqk(cc)
        return run

    inj.append(mk_k1_load(8))
    for cc in range(8, 16):
        if cc + 1 < 16:
            inj.append(mk_k1_load(cc + 1))
        inj.append(lambda cc=cc: make_qk_half(cc, 1, 0, k1_w[cc]))
        inj.append(lambda cc=cc: make_qk_half(cc, 1, 1, k1_w[cc]))
    n_k1_tasks = len(inj)

    # slot tail pieces; jst = [j, att_j, xq, x2t] then += [attt, ln2]
    def tail_attt(jst):
        j, att_j = jst[0], jst[1]
        attt = attt_pool.tile([128, 8, 128], BF16, tag="attt", name="attt")
        jst.append(attt)
        for g2 in range(2):
            pt = u_tile(BF16)
            for u in range(4):
                dc = g2 * 4 + u
                nc.tensor.transpose(
                    pt[:, u * 128:(u + 1) * 128],
                    att_j[:, dc * 128:(dc + 1) * 128], nc._ident[:])
            nc.vector.tensor_copy(attt[:, g2 * 4:(g2 + 1) * 4, :], pt[:])

    def tail_proj_half(jst, half):
        j, att_j, xq, x2t, attt = jst[0], jst[1], jst[2], jst[3], jst[4]
        ps = u_tile()
        for hcc in range(8):
            nc.tensor.matmul(
                ps[:], attt[:, hcc, :],
                PW[:, hcc, half * 512:(half + 1) * 512],
                start=(hcc == 0), stop=(hcc == 7),
            )
        sl = slice(half * 512, (half + 1) * 512)
        nc.vector.tensor_add(out=x2t[:, sl], in0=ps[:], in1=st["pb"][:, sl])
        nc.vector.tensor_add(out=x2t[:, sl], in0=x2t[:, sl], in1=xq[:, sl])

    def tail_x2out(jst):
        j, x2t = jst[0], jst[3]
        nc.sync.dma_start(x2_d.ap()[j * 128:(j + 1) * 128, :], x2t[:])
        ln2 = ln2p.tile([128, D], BF16, tag="ln2", name="ln2")
        rstd, nmr = _ln_stats(nc, epp, x2t[:], eps_t)
        nc.vector.tensor_scalar(
            out=ln2[:], in0=x2t[:], scalar1=rstd, scalar2=nmr[:],
            op0=ALU.mult, op1=ALU.add)
        jst.append(ln2)

    def tail_xt2(jst, g2):
        j, ln2 = jst[0], jst[5]
        pt = u_tile(BF16)
        for u in range(4):
            dc = g2 * 4 + u
            nc.tensor.transpose(
                pt[:, u * 128:(u + 1) * 128],
                ln2[:, dc * 128:(dc + 1) * 128], nc._ident[:])
        for u in range(4):
            dc = g2 * 4 + u
            nc.vector.tensor_scalar(
                out=XT2[:, dc, j * 128:(j + 1) * 128],
                in0=pt[:, u * 128:(u + 1) * 128],
                scalar1=st["g2"][:, dc:dc + 1], scalar2=st["b2"][:, dc:dc + 1],
                op0=ALU.mult, op1=ALU.add)

    w1_tiles = []

    def mk_w1_load(hh):
        def run():
            w1 = w1p.tile([128, 8, 128], BF16, tag="w1")
            nc.gpsimd.dma_start(
                w1[:], dram["wff1"].ap()[:, :, hh * 128:(hh + 1) * 128])
            w1_tiles.append(w1)
        return run

    # ---- attention stages ------------------------------------------------
    def emit_scores(j, h, att_j):
        hc, po = h >> 1, 64 * (h & 1)
        nblk = NKMAX[j]
        groups = []
        for g in range((nblk + 7) // 8):
            blo = g * 8
            bhi = min(blo + 8, nblk)
            ncol = (bhi - blo) * 128
            ps = sc_tile()
            for kk in range(blo, bhi):
                nc.tensor.matmul(
                    ps[:, (kk - blo) * 128:(kk - blo + 1) * 128],
                    KT[po:po + 64, hc, kk * 128:(kk + 1) * 128],
                    QT[po:po + 64, hc, j * 128:(j + 1) * 128],
                    start=True, stop=True,
                )
            at = atp.tile([128, 1024], BF16, tag="at", name="at")
            nc.scalar.activation(out=at[:, 0:ncol], in_=ps[:, 0:ncol],
                                 func=AF.Exp, scale=0.125)
            for kk in range(max(blo, nblk - 2), bhi):
                mc = (kk - (nblk - 2)) * 128
                nc.gpsimd.tensor_mul(
                    out=at[:, (kk - blo) * 128:(kk - blo + 1) * 128],
                    in0=at[:, (kk - blo) * 128:(kk - blo + 1) * 128],
                    in1=st["mask"][:, j, mc:mc + 128],
                )
            groups.append((blo, bhi, at))
        return groups

    def emit_pv(pend):
        h, j, groups, att_j = pend
        nblk = NKMAX[j]
        pv = u_tile()
        for blo, bhi, at in groups:
            for kk in range(blo, bhi):
                nc.tensor.matmul(
                    pv[:, 0:HD + 1],
                    at[:, (kk - blo) * 128:(kk - blo + 1) * 128],
                    V[:, kk, h, :],
                    start=(kk == 0), stop=(kk == nblk - 1),
                )
        r = epp.tile([128, 1], F32, tag="recip", name="r")
        nc.vector.reciprocal(out=r[:], in_=pv[:, HD:HD + 1])
        nc.vector.tensor_scalar_mul(
            out=att_j[:, h * HD:(h + 1) * HD],
            in0=pv[:, 0:HD], scalar1=r[:],
        )

    deferred = []

    def queue_tail(jst):
        # xt2 transposes of the PREVIOUS slot are queued behind this slot's
        # tail so the PE doesn't catch up with the DVE ln2 chain
        inj.append(lambda: tail_attt(jst))
        inj.append(lambda: tail_proj_half(jst, 0))
        inj.append(lambda: tail_proj_half(jst, 1))
        inj.append(lambda: tail_x2out(jst))
        inj.extend(deferred)
        deferred.clear()
        deferred.append(lambda: tail_xt2(jst, 0))
        deferred.append(lambda: tail_xt2(jst, 1))

    pending = None
    xtf_released = False
    k1_left = n_k1_tasks
    stage_i = 0
    for j in reversed(range(NQ)):
        att_j = att_pool.tile([128, D], BF16, tag="att", name="att_j")
        # prefetch this slot's xq + allocate its x2t up front
        xq = xqp.tile([128, D], F32, tag="xq", name="xq")
        nc.sync.dma_start(xq[:], xq_d.ap()[j * 128:(j + 1) * 128, :])
        x2t = x2t_pool.tile([128, D], F32, tag="x2t", name="x2t")
        jst = [j, att_j, xq, x2t]
        for h in range(H):
            groups = emit_scores(j, h, att_j)
            if pending is not None:
                emit_pv(pending)
            pending = (h, j, groups, att_j)
            stage_i += 1
            # drain K1 aggressively (3/stage) so XTF can release early;
            # afterwards 1-2 tasks per stage
            npop = 3 if k1_left > 0 else (
                2 if len(inj) > 8 and stage_i % 2 == 0 else 1)
            for _ in range(min(npop, len(inj))):
                inj.pop(0)()
                k1_left -= 1
            if k1_left <= 0 and not xtf_released:
                wqk_p.release()
                xtf_p.release()
                xtf_released = True
            if stage_i == 100:
                for hh in range(3):
                    inj.append(mk_w1_load(hh))
        queue_tail(jst)
    emit_pv(pending)
    while inj:
        inj.pop(0)()
    for f in deferred:
        f()

    wpp.release()
    ln2p.release()
    xqp.release()
    x2t_pool.release()
    att_pool.release()
    attt_pool.release()
    epp.release()
    atp.release()
    qkv_p.release()
    if stop_after == "p5":
        xtmp_p.release()
        u_ps.release()
        sc_ps.release()
        w1p.release()
        return

    # =====================================================================
    # FFN
    # =====================================================================
    ht_p = tc.alloc_tile_pool(name="ht", bufs=1)
    HT = ht_p.tile([128, 32, TQ], BF16, name="HT", tag="ht")
    X2F = ht_p.tile([128, NQ, D], F32, name="X2F", tag="x2f")
    with tc.tile_pool(name="x2f_ld", bufs=3) as x2fp:
        for t in range(NQ):
            x2l = x2fp.tile([128, D], F32, tag="x2l", name="x2l")
            nc.sync.dma_start(x2l[:], x2_d.ap()[t * 128:(t + 1) * 128, :])
            nc.vector.tensor_add(out=X2F[:, t, :], in0=x2l[:], in1=st["fb2"][:])

    for hh in range(32):
        if hh >= len(w1_tiles):
            mk_w1_load(hh)()
        w1 = w1_tiles[hh]
        ps = sc_tile()
        for half in range(2):
            for dc in range(8):
                nc.tensor.matmul(
                    ps[:, half * 512:(half + 1) * 512],
                    w1[:, dc, :],
                    XT2[:, dc, half * 512:(half + 1) * 512],
                    start=(dc == 0), stop=(dc == 7),
                )
        nc.scalar.activation(
            out=HT[:, hh, :], in_=ps[:], func=AF.Gelu,
            bias=st["bff1"][:, hh:hh + 1], scale=1.0)
    xtmp_p.release()
    u_ps.release()
    sc_ps.release()
    if stop_after == "ff1":
        ht_p.release()
        w1p.release()
        return

    with tc.tile_pool(name="w2", bufs=6) as w2p, \
         tc.tile_pool(name="yp", bufs=3) as yp, \
         tc.tile_pool(name="ff2ps", bufs=4, space=PSUM) as ff2ps:
        for tg in range(2):
            pss = [ff2ps.tile([128, 1024], F32, name="ym", tag="ym")
                   for _ in range(4)]
            for hh in range(32):
                w2 = w2p.tile([128, D], BF16, tag="w2")
                nc.gpsimd.dma_start(w2[:], dram["wff2"].ap()[:, hh, :])
                for tt in range(4):
                    t = tg * 4 + tt
                    for half in range(2):
                        nc.tensor.matmul(
                            pss[tt][:, half * 512:(half + 1) * 512],
                            HT[:, hh, t * 128:(t + 1) * 128],
                            w2[:, half * 512:(half + 1) * 512],
                            start=(hh == 0), stop=(hh == 31),
                        )
            for tt in range(4):
                t = tg * 4 + tt
                yt = yp.tile([128, D], F32, tag="yt")
                nc.vector.tensor_add(out=yt[:], in0=pss[tt][:], in1=X2F[:, t, :])
                nc.sync.dma_start(
                    y_d.ap()[t * 128:(t + 1) * 128, :], yt[:])
    ht_p.release()
    w1p.release()


# ---------------------------------------------------------------------------
# Host wrapper
# ---------------------------------------------------------------------------

_PROG_CACHE = {}


def _get_program(niter=None):
    if niter not in _PROG_CACHE:
        _PROG_CACHE[niter] = build_program(niter)
    return _PROG_CACHE[niter]


def make_in_maps(x, ln1_g, ln1_b, qkv_w, qkv_b, proj_w, proj_b,
                 ln2_g, ln2_b, ff1_w, ff1_b, ff2_w, ff2_b):
    bf = ml_dtypes.bfloat16
    f32 = np.float32

    def pcol(v, n):  # [n*128] -> [128, n] (chunk c holds elements c*128..c*128+127)
        return np.ascontiguousarray(np.asarray(v, f32).reshape(n, 128).T)

    def dimmajor(w, nchunk, ncol):  # [nchunk*128, ncol] -> [128, nchunk, ncol]
        return np.ascontiguousarray(
            np.asarray(w, f32).reshape(nchunk, 128, ncol).transpose(1, 0, 2)
        ).astype(bf)

    qkv_b = np.asarray(qkv_b, f32)
    common = dict(
        wqkv=dimmajor(qkv_w, 8, 3 * D),
        wproj=dimmajor(proj_w, 8, D),
        wff1=dimmajor(ff1_w, 8, HID),
        wff2=dimmajor(ff2_w, 32, D),
        bqk=np.ascontiguousarray(
            np.concatenate([pcol(qkv_b[0:D], 8), pcol(qkv_b[D:2 * D], 8)], axis=1)),
        bv=qkv_b[2 * D:3 * D].copy(),
        bproj=np.asarray(proj_b, f32).copy(),
        bff1=pcol(ff1_b, 32),
        bff2=np.asarray(ff2_b, f32).copy(),
        g1=pcol(ln1_g, 8), b1=pcol(ln1_b, 8),
        g2=pcol(ln2_g, 8), b2=pcol(ln2_b, 8),
    )
    masks = [np.ascontiguousarray(_masks(p).transpose(1, 0, 2)).astype(bf)
             for p in range(2)]
    perms = [_perm(0), _perm(1)]

    x = np.asarray(x, f32)
    in_maps = []
    for c in range(N_CORES):
        b, p = c // 2, c % 2
        m = dict(common)
        m["x_full"] = np.ascontiguousarray(x[b])
        m["x_q"] = np.ascontiguousarray(x[b][perms[p]])
        m["mask"] = masks[p]
        in_maps.append(m)
    return in_maps, perms


def kernel(**inputs):
    in_maps, perms = make_in_maps(**{k: np.asarray(v) for k, v in inputs.items()})
    nc = _get_program()
    res = run_bass_kernel_spmd(nc, in_maps, list(range(N_CORES))).results
    y = np.empty((B, T, D), np.float32)
    for c in range(N_CORES):
        b, p = c // 2, c % 2
        y[b][perms[p]] = res[c]["y"]
    return y


# revision 18
# speedup vs baseline: 1.2300x; 1.2300x over previous
"""Trainium2 Bass kernel for a dense transformer block (B=4, T=2048, D=1024, H=16).

Sharding: 8 cores = 4 batches x 2 query-parity groups. Each core computes
LN1+QKV (K/V for the full sequence, all heads) for its batch, then causal
attention + proj + FFN for its half of the query blocks only. Query blocks are
assigned in a zigzag so causal attention work is balanced across the two cores
of a batch:
  parity 0 -> blocks [15,12,11,8,7,4,3,0]   (causal lengths 16,13,12,9,8,5,4,1)
  parity 1 -> blocks [14,13,10,9,6,5,2,1]   (causal lengths 15,14,11,10,7,6,3,2)
Slot j on every core processes NKMAX[j] = [16,14,12,10,8,6,4,2] key blocks, so
the SPMD program is identical on all 8 cores; the causal mask for the last two
key blocks of each slot is an input (differs by parity). The host permutes each
core's query tokens into slot order and inverse-permutes the output.

Matmul operands are bf16 (fp32 PE matmul is 4x slower); accumulation, softmax
denominators, layernorm stats and residuals are fp32. Softmax skips the
max-subtraction (scores are ~N(0,1) after the 1/8 scale; exp is exact softmax
mathematically and safe in fp32/bf16 range here), which lets the denominator
come from an appended ones-column in V during the PV matmul.

The PE is in-order, so emission order is the schedule. The body is one
software-pipelined stream: LN1 tiles feed transposes feed QKV matmuls at
token-block granularity; attention stages (one head x one slot) interleave
score matmuls, exp (ACT), mask (Pool/GpSimd), and PV so the PE never idles on
the exp drain; K-pair-1 production, per-slot proj, LN2 and XT2 transposes are
injected between stages as PE filler. FFN weights prefetch during the
attention tail.
"""

import sys

try:
    import concourse  # noqa: F401
except ImportError:
    sys.path.insert(0, "/opt/trn_rl_repo")

import numpy as np
import ml_dtypes

import concourse.bass as bass
import concourse.bacc as bacc
import concourse.tile as tile
from concourse import mybir
from concourse.bass_utils import run_bass_kernel_spmd

F32 = mybir.dt.float32
BF16 = mybir.dt.bfloat16
AF = mybir.ActivationFunctionType
ALU = mybir.AluOpType
PSUM = bass.MemorySpace.PSUM

B, T, D = 4, 2048, 1024
H, HD = 16, 64
HID = 4 * D
EPS = 1e-5
N_CORES = 8
NB = T // 128  # 16 token blocks per batch
NQ = 8  # query slots per core
TQ = NQ * 128  # query tokens per core

QB = [
    [15, 12, 11, 8, 7, 4, 3, 0],
    [14, 13, 10, 9, 6, 5, 2, 1],
]
NKMAX = [16, 14, 12, 10, 8, 6, 4, 2]


def _perm(p):
    idx = []
    for qb in QB[p]:
        idx.extend(range(qb * 128, (qb + 1) * 128))
    return np.array(idx, dtype=np.int64)


def _masks(p):
    """[NQ, 128, 256] multiplicative mask for the last two key blocks per slot."""
    m = np.zeros((NQ, 128, 256), dtype=np.float32)
    tri = np.triu(np.ones((128, 128), np.float32))  # valid where k_local <= q_local
    for j in range(NQ):
        nk = QB[p][j] + 1
        nkm = NKMAX[j]
        for c in range(2):
            ki = nkm - 2 + c
            if ki < nk - 1:
                m[j, :, c * 128:(c + 1) * 128] = 1.0
            elif ki == nk - 1:
                m[j, :, c * 128:(c + 1) * 128] = tri
    return m


# ---------------------------------------------------------------------------
# Device program
# ---------------------------------------------------------------------------

def _ln_stats(nc, pool, x_ap, eps_t):
    """LN stats of a [128, D] fp32 tile -> (rstd, nmr) [128,1] tiles."""
    stats = pool.tile([128, 2, nc.vector.BN_STATS_DIM], F32, tag="ln_stats")
    mv = pool.tile([128, nc.vector.BN_AGGR_DIM], F32, tag="ln_mv")
    nmr = pool.tile([128, 1], F32, tag="ln_nmr")
    xg = x_ap.rearrange("p (s d) -> p s d", s=2)
    for s in range(2):
        nc.vector.bn_stats(out=stats[:, s, :], in_=xg[:, s, :])
    nc.vector.bn_aggr(out=mv[:], in_=stats[:])
    mean = mv[:, 0:1]
    rstd = mv[:, 1:2]
    nc.scalar.activation(out=rstd, in_=rstd, func=AF.Sqrt, bias=eps_t[:], scale=1.0)
    nc.vector.reciprocal(out=rstd, in_=rstd)
    nc.vector.tensor_scalar(
        out=nmr[:], in0=mean, scalar1=rstd, scalar2=-1.0,
        op0=ALU.mult, op1=ALU.mult,
    )
    return rstd, nmr


def build_program(niter=None, stop_after=None):
    nc = bacc.Bacc("TRN2", target_bir_lowering=False, debug=False,
                   num_devices=N_CORES)

    xf_d = nc.dram_tensor("x_full", [T, D], F32, kind="ExternalInput")
    xq_d = nc.dram_tensor("x_q", [TQ, D], F32, kind="ExternalInput")
    mask_d = nc.dram_tensor("mask", [128, NQ, 256], BF16, kind="ExternalInput")
    wqkv_d = nc.dram_tensor("wqkv", [128, 8, 3 * D], BF16, kind="ExternalInput")
    wproj_d = nc.dram_tensor("wproj", [128, 8, D], BF16, kind="ExternalInput")
    wff1_d = nc.dram_tensor("wff1", [128, 8, HID], BF16, kind="ExternalInput")
    wff2_d = nc.dram_tensor("wff2", [128, 32, D], BF16, kind="ExternalInput")
    bqk_d = nc.dram_tensor("bqk", [128, 16], F32, kind="ExternalInput")
    bv_d = nc.dram_tensor("bv", [D], F32, kind="ExternalInput")
    bproj_d = nc.dram_tensor("bproj", [D], F32, kind="ExternalInput")
    bff1_d = nc.dram_tensor("bff1", [128, 32], F32, kind="ExternalInput")
    bff2_d = nc.dram_tensor("bff2", [D], F32, kind="ExternalInput")
    g1_d = nc.dram_tensor("g1", [128, 8], F32, kind="ExternalInput")
    b1_d = nc.dram_tensor("b1", [128, 8], F32, kind="ExternalInput")
    g2_d = nc.dram_tensor("g2", [128, 8], F32, kind="ExternalInput")
    b2_d = nc.dram_tensor("b2", [128, 8], F32, kind="ExternalInput")
    y_d = nc.dram_tensor("y", [TQ, D], F32, kind="ExternalOutput")

    def bcast_row(dram_handle, n):
        ap = dram_handle.ap()
        return bass.AP(tensor=ap.tensor, offset=ap.offset, ap=[[0, 128], [1, n]])

    dram = dict(xf=xf_d, xq=xq_d, wqkv=wqkv_d, wproj=wproj_d,
                wff1=wff1_d, wff2=wff2_d, y=y_d)

    with tile.TileContext(nc) as tc:
        with tc.tile_pool(name="singles", bufs=1) as singles:
            ident = singles.tile([128, 128], BF16)
            from concourse.masks import make_identity
            make_identity(nc, ident[:])
            nc._ident = ident

            st = {}
            st["eps"] = singles.tile([128, 1], F32, name="eps", tag="eps")
            nc.vector.memset(st["eps"][:], EPS)
            for nm, dt, shape, src in (
                ("g1", F32, [128, 8], g1_d), ("b1", F32, [128, 8], b1_d),
                ("g2", F32, [128, 8], g2_d), ("b2", F32, [128, 8], b2_d),
                ("bqk", F32, [128, 16], bqk_d), ("bff1", F32, [128, 32], bff1_d),
                ("mask", BF16, [128, NQ, 256], mask_d),
            ):
                st[nm] = singles.tile(shape, dt, name=nm, tag=nm)
                nc.sync.dma_start(st[nm][:], src.ap())
            for nm, src in (("vb", bv_d), ("pb", bproj_d), ("fb2", bff2_d)):
                st[nm] = singles.tile([128, D], F32, name=nm, tag=nm)
                nc.gpsimd.dma_start(st[nm][:], bcast_row(src, D))

            if niter is None:
                _block_body(tc, nc, dram, st, stop_after)
            else:
                with tc.For_i(0, niter, 1):
                    _block_body(tc, nc, dram, st, stop_after)

    nc.compile()
    return nc


def _block_body(tc, nc, dram, st, stop_after=None):
    xf_d, xq_d, y_d = dram["xf"], dram["xq"], dram["y"]
    eps_t = st["eps"]
    x2_d = nc.dram_tensor("x2_scratch", [TQ, D], F32, kind="Internal")

    # --- persistent SBUF (right side, LIFO release order) -----------------
    # XTMP serves as XTQ during the prefix and is recycled as XT2 during
    # attention (XTQ's last read precedes XT2's first write; tile WAR deps
    # order them).  w1p sits at the bottom of the left stack so it can
    # outlive the attention pools into ff1.
    w1p = tc.alloc_tile_pool(name="w1", bufs=4)
    xtmp_p = tc.alloc_tile_pool(name="xtmp", bufs=1, side="right")
    XTQ = xtmp_p.tile([128, 8, TQ], BF16, name="XTMP", tag="xtmp")
    XT2 = XTQ
    qkv_p = tc.alloc_tile_pool(name="qkv", bufs=1, side="right")
    QT = qkv_p.tile([128, 8, TQ], BF16, name="QT", tag="qt")
    KT = qkv_p.tile([128, 8, T], BF16, name="KT", tag="kt")
    V = qkv_p.tile([128, NB, H, HD + 1], BF16, name="V", tag="v")
    for t in range(NB):
        nc.vector.memset(V[:, t, :, HD:HD + 1], 1.0)

    xtf_p = tc.alloc_tile_pool(name="xtf", bufs=1, side="right")
    XTF = xtf_p.tile([128, 8, T], BF16, name="XTF", tag="xtf")

    # --- PSUM pools: sc 3x[128,1024]f32 (6 banks) + u 2x2KB (2 banks) -----
    sc_ps = tc.alloc_tile_pool(name="sc_ps", bufs=2, space=PSUM)
    u_ps = tc.alloc_tile_pool(name="u_ps", bufs=4, space=PSUM)

    def sc_tile():
        return sc_ps.tile([128, 1024], F32, tag="sc", name="ps")

    def u_tile(dt=F32):
        if dt == F32:
            return u_ps.tile([128, 512], dt, tag="u", name="ups")
        return u_ps.tile([128, 512], dt, tag="u", name="ups",
                         padded_shape=[128, 1024])

    # --- prefix pools (right side, above xtf) -----------------------------
    wqk_p = tc.alloc_tile_pool(name="wqk", bufs=4, side="right")
    wv_p = tc.alloc_tile_pool(name="wv", bufs=2, side="right")
    p1 = tc.alloc_tile_pool(name="p1", bufs=5, side="right")
    p1s = tc.alloc_tile_pool(name="p1s", bufs=8, side="right")
    xln_p = tc.alloc_tile_pool(name="xln", bufs=4, side="right")

    # ---- LN1 pipeline pieces --------------------------------------------
    def ln1_tile(x_d, row0, chunks=1):
        """Load [128, D] from x_d at row0, LN -> bf16 tile (no gain yet).

        chunks>1 splits the load across DMA queues to cut arrival latency
        (matters for the first tiles after the For_i iteration barrier)."""
        xt = p1.tile([128, D], F32, tag="xt", name="xt")
        cw = D // chunks
        for c in range(chunks):
            nc.sync.dma_start(xt[:, c * cw:(c + 1) * cw],
                              x_d.ap()[row0:row0 + 128, c * cw:(c + 1) * cw])
        ln = xln_p.tile([128, D], BF16, tag="ln", name="ln")
        rstd, nmr = _ln_stats(nc, p1s, xt[:], eps_t)
        nc.scalar.activation(out=ln[:], in_=xt[:], func=AF.Identity,
                             bias=nmr[:], scale=rstd)
        return ln

    def tp_group(tiles4, dst3, col0):
        """Transpose 4 LN tiles' dim-chunks into dst3[:, dc, col0:col0+512].

        g1/b1 fused into the psum->sbuf copy (DVE even dc / ACT odd dc)."""
        for dc in range(8):
            pt = u_tile(BF16)
            for u in range(4):
                nc.tensor.transpose(
                    pt[:, u * 128:(u + 1) * 128],
                    tiles4[u][:, dc * 128:(dc + 1) * 128], nc._ident[:])
            dst = dst3[:, dc, col0:col0 + 512]
            if dc % 2 == 1:
                nc.scalar.activation(
                    out=dst, in_=pt[:], func=AF.Identity,
                    bias=st["b1"][:, dc:dc + 1], scale=st["g1"][:, dc:dc + 1])
            else:
                nc.vector.tensor_scalar(
                    out=dst, in0=pt[:],
                    scalar1=st["g1"][:, dc:dc + 1], scalar2=st["b1"][:, dc:dc + 1],
                    op0=ALU.mult, op1=ALU.add)

    def make_qk_half(cc, pair, h2, w):
        """One [128,512] half of a QT (cc<8) / KT (cc>=8) chunk via u_ps."""
        is_q = cc < 8
        XT = XTQ if is_q else XTF
        OUT = QT if is_q else KT
        occ = cc if is_q else cc - 8
        ps = u_tile()
        for dc in range(8):
            nc.tensor.matmul(
                ps[:], w[:, dc, :],
                XT[:, dc, (pair * 2 + h2) * 512:(pair * 2 + h2 + 1) * 512],
                start=(dc == 0), stop=(dc == 7),
            )
        nc.vector.tensor_scalar_add(
            out=OUT[:, occ, (pair * 2 + h2) * 512:(pair * 2 + h2 + 1) * 512],
            in0=ps[:], scalar1=st["bqk"][:, cc:cc + 1],
        )

    def load_wqk(cc):
        w = wqk_p.tile([128, 8, 128], BF16, tag="wqk", name="w")
        nc.sync.dma_start(
            w[:], dram["wqkv"].ap()[:, :, cc * 128:(cc + 1) * 128])
        return w

    def v_block(t, vh, wv):
        """V for token block t, head-half vh (8 heads x 64) via u_ps."""
        ps = u_tile()
        for dc in range(8):
            nc.tensor.matmul(
                ps[:], XTF[:, dc, t * 128:(t + 1) * 128],
                wv[:, dc, :], start=(dc == 0), stop=(dc == 7),
            )
        nc.vector.tensor_add(
            out=V[:, t, vh * 8:(vh + 1) * 8, 0:HD],
            in0=ps[:].rearrange("p (h d) -> p h d", h=8),
            in1=st["vb"][:, vh * 512:(vh + 1) * 512]
                .rearrange("p (h d) -> p h d", h=8),
        )

    # =====================================================================
    # PREFIX: pipelined LN1 -> transposes -> K0 / Q / V.  Emission order is
    # the PE schedule; LN for later groups overlaps earlier PE matmuls.
    # =====================================================================
    wv0 = wv_p.tile([128, 8, 512], BF16, tag="wv", name="wv0")
    nc.sync.dma_start(wv0[:], dram["wqkv"].ap()[:, :, 2 * D:2 * D + 512])
    wv1 = wv_p.tile([128, 8, 512], BF16, tag="wv", name="wv1")
    nc.sync.dma_start(wv1[:], dram["wqkv"].ap()[:, :, 2 * D + 512:3 * D])

    # xf group 0 (blocks 0..3) -> XTF cols 0..512 (chunked loads: these
    # gate the whole pipeline right after the iteration barrier)
    tiles = [ln1_tile(xf_d, u * 128, chunks=4) for u in range(4)]
    tp_group(tiles, XTF, 0)

    # K pair-0 lower halves (tokens 0..512) on PE; xf group 1 + xq LN
    # tiles stream on ACT/DVE meanwhile.  Weight chunks are re-loaded per
    # half-pass (cheap: gpsimd-issued DMA) to keep the wqk pool at 3 bufs.
    ln_feed = [lambda u=u: ln1_tile(xf_d, (4 + u) * 128) for u in range(4)]
    ln_feed += [lambda u=u: ln1_tile(xq_d, u * 128) for u in range(8)]
    ln_out = []

    def feed(n):
        for _ in range(n):
            if ln_feed:
                ln_out.append(ln_feed.pop(0)())

    wk = [load_wqk(8), load_wqk(9)]
    for i, cc in enumerate(range(8, 16)):
        if cc + 2 < 16:
            wk.append(load_wqk(cc + 2))
        feed(1)
        make_qk_half(cc, 0, 0, wk[i])
    tp_group(ln_out[0:4], XTF, 512)  # xf group 1

    # K pair-0 upper halves (tokens 512..1024); remaining xq LN streams
    wk = [load_wqk(8), load_wqk(9)]
    for i, cc in enumerate(range(8, 16)):
        if cc + 2 < 16:
            wk.append(load_wqk(cc + 2))
        feed(1)
        make_qk_half(cc, 0, 1, wk[i])
    tp_group(ln_out[4:8], XTQ, 0)
    tp_group(ln_out[8:12], XTQ, 512)

    # Q (all chunks, both halves); xf group 2 LN meanwhile
    ln_feed = [lambda u=u: ln1_tile(xf_d, (8 + u) * 128) for u in range(8)]
    ln_out = []
    wk = [load_wqk(0), load_wqk(1)]
    for cc in range(8):
        if cc + 2 < 8:
            wk.append(load_wqk(cc + 2))
        feed(1)
        make_qk_half(cc, 0, 0, wk[cc])
        make_qk_half(cc, 0, 1, wk[cc])

    # V blocks 0..7; xf group 3 LN meanwhile
    for t in range(8):
        feed(1)
        v_block(t, 0, wv0)
        v_block(t, 1, wv1)
    # xf groups 2,3 transposes -> XTF cols 1024..2048
    tp_group(ln_out[0:4], XTF, 1024)
    tp_group(ln_out[4:8], XTF, 1536)
    # V blocks 8..15
    for t in range(8, 16):
        v_block(t, 0, wv0)
        v_block(t, 1, wv1)

    xln_p.release()
    p1s.release()
    p1.release()
    wv_p.release()
    if stop_after == "prefix":
        wqk_p.release()
        xtf_p.release()
        qkv_p.release()
        xtmp_p.release()
        u_ps.release()
        sc_ps.release()
        w1p.release()
        return

    # =====================================================================
    # ATTENTION: stages = (slot j desc, head h asc). Injected between
    # stages: K pair 1 halves, per-slot proj/LN2/XT2, weight prefetch.
    # =====================================================================
    atp = tc.alloc_tile_pool(name="at", bufs=4)
    epp = tc.alloc_tile_pool(name="ep", bufs=4)
    attt_pool = tc.alloc_tile_pool(name="atttsl", bufs=2)
    att_pool = tc.alloc_tile_pool(name="attsl", bufs=2)
    x2t_pool = tc.alloc_tile_pool(name="x2tp", bufs=1)
    xqp = tc.alloc_tile_pool(name="xqp", bufs=1)
    ln2p = tc.alloc_tile_pool(name="ln2p", bufs=2)
    wpp = tc.alloc_tile_pool(name="wproj", bufs=1)

    PW = wpp.tile([128, 8, D], BF16, name="PW", tag="pw")
    nc.sync.dma_start(PW[:], dram["wproj"].ap())

    # ---- injected tasks (popped 1-2 per stage, FIFO) --------------------
    inj = []
    k1_w = {}

    def mk_k1_load(cc):
        def run():
            k1_w[cc] = load_wqk(cc)
        return run

    inj.append(mk_k1_load(8))
    for cc in range(8, 16):
        if cc + 1 < 16:
            inj.append(mk_k1_load(cc + 1))
        inj.append(lambda cc=cc: make_qk_half(cc, 1, 0, k1_w[cc]))
        inj.append(lambda cc=cc: make_qk_half(cc, 1, 1, k1_w[cc]))
    n_k1_tasks = len(inj)

    # slot tail pieces; jst = [j, att_j, xq, x2t] then += [attt, ln2]
    def tail_attt(jst):
        j, att_j = jst[0], jst[1]
        attt = attt_pool.tile([128, 8, 128], BF16, tag="attt", name="attt")
        jst.append(attt)
        for g2 in range(2):
            pt = u_tile(BF16)
            for u in range(4):
                dc = g2 * 4 + u
                nc.tensor.transpose(
                    pt[:, u * 128:(u + 1) * 128],
                    att_j[:, dc * 128:(dc + 1) * 128], nc._ident[:])
            nc.vector.tensor_copy(attt[:, g2 * 4:(g2 + 1) * 4, :], pt[:])

    def tail_proj_half(jst, half):
        j, att_j, xq, x2t, attt = jst[0], jst[1], jst[2], jst[3], jst[4]
        ps = u_tile()
        for hcc in range(8):
            nc.tensor.matmul(
                ps[:], attt[:, hcc, :],
                PW[:, hcc, half * 512:(half + 1) * 512],
                start=(hcc == 0), stop=(hcc == 7),
            )
        sl = slice(half * 512, (half + 1) * 512)
        nc.vector.tensor_add(out=x2t[:, sl], in0=ps[:], in1=st["pb"][:, sl])
        nc.vector.tensor_add(out=x2t[:, sl], in0=x2t[:, sl], in1=xq[:, sl])

    def tail_x2out(jst):
        j, x2t = jst[0], jst[3]
        nc.sync.dma_start(x2_d.ap()[j * 128:(j + 1) * 128, :], x2t[:])
        ln2 = ln2p.tile([128, D], BF16, tag="ln2", name="ln2")
        rstd, nmr = _ln_stats(nc, epp, x2t[:], eps_t)
        nc.vector.tensor_scalar(
            out=ln2[:], in0=x2t[:], scalar1=rstd, scalar2=nmr[:],
            op0=ALU.mult, op1=ALU.add)
        jst.append(ln2)

    def tail_xt2(jst, g2):
        j, ln2 = jst[0], jst[5]
        pt = u_tile(BF16)
        for u in range(4):
            dc = g2 * 4 + u
            nc.tensor.transpose(
                pt[:, u * 128:(u + 1) * 128],
                ln2[:, dc * 128:(dc + 1) * 128], nc._ident[:])
        for u in range(4):
            dc = g2 * 4 + u
            nc.vector.tensor_scalar(
                out=XT2[:, dc, j * 128:(j + 1) * 128],
                in0=pt[:, u * 128:(u + 1) * 128],
                scalar1=st["g2"][:, dc:dc + 1], scalar2=st["b2"][:, dc:dc + 1],
                op0=ALU.mult, op1=ALU.add)

    w1_tiles = []

    def mk_w1_load(hh):
        def run():
            w1 = w1p.tile([128, 8, 128], BF16, tag="w1")
            nc.sync.dma_start(
                w1[:], dram["wff1"].ap()[:, :, hh * 128:(hh + 1) * 128])
            w1_tiles.append(w1)
        return run

    # ---- attention stages ------------------------------------------------
    def emit_scores(j, h, att_j):
        hc, po = h >> 1, 64 * (h & 1)
        nblk = NKMAX[j]
        groups = []
        for g in range((nblk + 7) // 8):
            blo = g * 8
            bhi = min(blo + 8, nblk)
            ncol = (bhi - blo) * 128
            ps = sc_tile()
            for kk in range(blo, bhi):
                nc.tensor.matmul(
                    ps[:, (kk - blo) * 128:(kk - blo + 1) * 128],
                    KT[po:po + 64, hc, kk * 128:(kk + 1) * 128],
                    QT[po:po + 64, hc, j * 128:(j + 1) * 128],
                    start=True, stop=True,
                )
            at = atp.tile([128, 1024], BF16, tag="at", name="at")
            nc.scalar.activation(out=at[:, 0:ncol], in_=ps[:, 0:ncol],
                                 func=AF.Exp, scale=0.125)
            for kk in range(max(blo, nblk - 2), bhi):
                mc = (kk - (nblk - 2)) * 128
                nc.vector.tensor_mul(
                    out=at[:, (kk - blo) * 128:(kk - blo + 1) * 128],
                    in0=at[:, (kk - blo) * 128:(kk - blo + 1) * 128],
                    in1=st["mask"][:, j, mc:mc + 128],
                )
            groups.append((blo, bhi, at))
        return groups

    def emit_pv(pend):
        h, j, groups, att_j = pend
        nblk = NKMAX[j]
        pv = u_tile()
        for blo, bhi, at in groups:
            for kk in range(blo, bhi):
                nc.tensor.matmul(
                    pv[:, 0:HD + 1],
                    at[:, (kk - blo) * 128:(kk - blo + 1) * 128],
                    V[:, kk, h, :],
                    start=(kk == 0), stop=(kk == nblk - 1),
                )
        r = epp.tile([128, 1], F32, tag="recip", name="r")
        nc.vector.reciprocal(out=r[:], in_=pv[:, HD:HD + 1])
        nc.vector.tensor_scalar_mul(
            out=att_j[:, h * HD:(h + 1) * HD],
            in0=pv[:, 0:HD], scalar1=r[:],
        )

    deferred = []

    def queue_tail(jst):
        # xt2 transposes of the PREVIOUS slot are queued behind this slot's
        # tail so the PE doesn't catch up with the DVE ln2 chain
        inj.append(lambda: tail_attt(jst))
        inj.append(lambda: tail_proj_half(jst, 0))
        inj.append(lambda: tail_proj_half(jst, 1))
        inj.append(lambda: tail_x2out(jst))
        inj.extend(deferred)
        deferred.clear()
        deferred.append(lambda: tail_xt2(jst, 0))
        deferred.append(lambda: tail_xt2(jst, 1))

    pending = None
    xtf_released = False
    k1_left = n_k1_tasks
    stage_i = 0
    for j in reversed(range(NQ)):
        att_j = att_pool.tile([128, D], BF16, tag="att", name="att_j")
        # prefetch this slot's xq + allocate its x2t up front
        xq = xqp.tile([128, D], F32, tag="xq", name="xq")
        nc.sync.dma_start(xq[:], xq_d.ap()[j * 128:(j + 1) * 128, :])
        x2t = x2t_pool.tile([128, D], F32, tag="x2t", name="x2t")
        jst = [j, att_j, xq, x2t]
        for h in range(H):
            groups = emit_scores(j, h, att_j)
            if pending is not None:
                emit_pv(pending)
            pending = (h, j, groups, att_j)
            stage_i += 1
            # drain K1 aggressively (3/stage) so XTF can release early;
            # afterwards 1-2 tasks per stage
            npop = 3 if k1_left > 0 else (
                2 if len(inj) > 8 and stage_i % 2 == 0 else 1)
            for _ in range(min(npop, len(inj))):
                inj.pop(0)()
                k1_left -= 1
            if k1_left <= 0 and not xtf_released:
                wqk_p.release()
                xtf_p.release()
                xtf_released = True
            if stage_i == 100:
                for hh in range(3):
                    inj.append(mk_w1_load(hh))
        queue_tail(jst)
    emit_pv(pending)
    while inj:
        inj.pop(0)()
    for f in deferred:
        f()

    wpp.release()
    ln2p.release()
    xqp.release()
    x2t_pool.release()
    att_pool.release()
    attt_pool.release()
    epp.release()
    atp.release()
    qkv_p.release()
    if stop_after == "p5":
        xtmp_p.release()
        u_ps.release()
        sc_ps.release()
        w1p.release()
        return

    # =====================================================================
    # FFN
    # =====================================================================
    ht_p = tc.alloc_tile_pool(name="ht", bufs=1)
    HT = ht_p.tile([128, 32, TQ], BF16, name="HT", tag="ht")
    X2F = ht_p.tile([128, NQ, D], F32, name="X2F", tag="x2f")
    with tc.tile_pool(name="x2f_ld", bufs=3) as x2fp:
        for t in range(NQ):
            x2l = x2fp.tile([128, D], F32, tag="x2l", name="x2l")
            nc.sync.dma_start(x2l[:], x2_d.ap()[t * 128:(t + 1) * 128, :])
            nc.vector.tensor_add(out=X2F[:, t, :], in0=x2l[:], in1=st["fb2"][:])

    for hh in range(32):
        if hh >= len(w1_tiles):
            mk_w1_load(hh)()
        w1 = w1_tiles[hh]
        ps = sc_tile()
        for half in range(2):
            for dc in range(8):
                nc.tensor.matmul(
                    ps[:, half * 512:(half + 1) * 512],
                    w1[:, dc, :],
                    XT2[:, dc, half * 512:(half + 1) * 512],
                    start=(dc == 0), stop=(dc == 7),
                )
        nc.scalar.activation(
            out=HT[:, hh, :], in_=ps[:], func=AF.Gelu,
            bias=st["bff1"][:, hh:hh + 1], scale=1.0)
    xtmp_p.release()
    u_ps.release()
    sc_ps.release()
    if stop_after == "ff1":
        ht_p.release()
        w1p.release()
        return

    with tc.tile_pool(name="w2", bufs=6) as w2p, \
         tc.tile_pool(name="yp", bufs=3) as yp, \
         tc.tile_pool(name="ff2ps", bufs=4, space=PSUM) as ff2ps:
        for tg in range(2):
            pss = [ff2ps.tile([128, 1024], F32, name="ym", tag="ym")
                   for _ in range(4)]
            for hh in range(32):
                w2 = w2p.tile([128, D], BF16, tag="w2")
                nc.sync.dma_start(w2[:], dram["wff2"].ap()[:, hh, :])
                for tt in range(4):
                    t = tg * 4 + tt
                    for half in range(2):
                        nc.tensor.matmul(
                            pss[tt][:, half * 512:(half + 1) * 512],
                            HT[:, hh, t * 128:(t + 1) * 128],
                            w2[:, half * 512:(half + 1) * 512],
                            start=(hh == 0), stop=(hh == 31),
                        )
            for tt in range(4):
                t = tg * 4 + tt
                yt = yp.tile([128, D], F32, tag="yt")
                nc.vector.tensor_add(out=yt[:], in0=pss[tt][:], in1=X2F[:, t, :])
                nc.sync.dma_start(
                    y_d.ap()[t * 128:(t + 1) * 128, :], yt[:])
    ht_p.release()
    w1p.release()


# ---------------------------------------------------------------------------
# Host wrapper
# ---------------------------------------------------------------------------

_PROG_CACHE = {}


def _get_program(niter=None):
    if niter not in _PROG_CACHE:
        _PROG_CACHE[niter] = build_program(niter)
    return _PROG_CACHE[niter]


def make_in_maps(x, ln1_g, ln1_b, qkv_w, qkv_b, proj_w, proj_b,
                 ln2_g, ln2_b, ff1_w, ff1_b, ff2_w, ff2_b):
    bf = ml_dtypes.bfloat16
    f32 = np.float32

    def pcol(v, n):  # [n*128] -> [128, n] (chunk c holds elements c*128..c*128+127)
        return np.ascontiguousarray(np.asarray(v, f32).reshape(n, 128).T)

    def dimmajor(w, nchunk, ncol):  # [nchunk*128, ncol] -> [128, nchunk, ncol]
        return np.ascontiguousarray(
            np.asarray(w, f32).reshape(nchunk, 128, ncol).transpose(1, 0, 2)
        ).astype(bf)

    qkv_b = np.asarray(qkv_b, f32)
    common = dict(
        wqkv=dimmajor(qkv_w, 8, 3 * D),
        wproj=dimmajor(proj_w, 8, D),
        wff1=dimmajor(ff1_w, 8, HID),
        wff2=dimmajor(ff2_w, 32, D),
        bqk=np.ascontiguousarray(
            np.concatenate([pcol(qkv_b[0:D], 8), pcol(qkv_b[D:2 * D], 8)], axis=1)),
        bv=qkv_b[2 * D:3 * D].copy(),
        bproj=np.asarray(proj_b, f32).copy(),
        bff1=pcol(ff1_b, 32),
        bff2=np.asarray(ff2_b, f32).copy(),
        g1=pcol(ln1_g, 8), b1=pcol(ln1_b, 8),
        g2=pcol(ln2_g, 8), b2=pcol(ln2_b, 8),
    )
    masks = [np.ascontiguousarray(_masks(p).transpose(1, 0, 2)).astype(bf)
             for p in range(2)]
    perms = [_perm(0), _perm(1)]

    x = np.asarray(x, f32)
    in_maps = []
    for c in range(N_CORES):
        b, p = c // 2, c % 2
        m = dict(common)
        m["x_full"] = np.ascontiguousarray(x[b])
        m["x_q"] = np.ascontiguousarray(x[b][perms[p]])
        m["mask"] = masks[p]
        in_maps.append(m)
    return in_maps, perms


def kernel(**inputs):
    in_maps, perms = make_in_maps(**{k: np.asarray(v) for k, v in inputs.items()})
    nc = _get_program()
    res = run_bass_kernel_spmd(nc, in_maps, list(range(N_CORES))).results
    y = np.empty((B, T, D), np.float32)
    for c in range(N_CORES):
        b, p = c // 2, c % 2
        y[b][perms[p]] = res[c]["y"]
    return y


# revision 28
# speedup vs baseline: 1.2962x; 1.0538x over previous
"""Trainium2 Bass kernel for a dense transformer block (B=4, T=2048, D=1024, H=16).

Sharding: 8 cores = 4 batches x 2 query-parity groups. Each core computes
LN1+QKV (K/V for the full sequence, all heads) for its batch, then causal
attention + proj + FFN for its half of the query blocks only. Query blocks are
assigned in a zigzag so causal attention work is balanced across the two cores
of a batch:
  parity 0 -> blocks [15,12,11,8,7,4,3,0]   (causal lengths 16,13,12,9,8,5,4,1)
  parity 1 -> blocks [14,13,10,9,6,5,2,1]   (causal lengths 15,14,11,10,7,6,3,2)
Slot j on every core processes NKMAX[j] = [16,14,12,10,8,6,4,2] key blocks, so
the SPMD program is identical on all 8 cores; the causal mask for the last two
key blocks of each slot is an input (differs by parity). The host permutes each
core's query tokens into slot order and inverse-permutes the output.

Matmul operands are bf16 (fp32 PE matmul is 4x slower); accumulation, softmax
denominators, layernorm stats and residuals are fp32. Softmax skips the
max-subtraction (scores are ~N(0,1) after the 1/8 scale; exp is exact softmax
mathematically and safe in fp32/bf16 range here), which lets the denominator
come from an appended ones-column in V during the PV matmul.

The PE is in-order, so emission order is the schedule. The body is one
software-pipelined stream: LN1 tiles feed transposes feed QKV matmuls at
token-block granularity; attention stages (one head x one slot) interleave
score matmuls, exp (ACT), mask (Pool/GpSimd), and PV so the PE never idles on
the exp drain; K-pair-1 production, per-slot proj, LN2 and XT2 transposes are
injected between stages as PE filler. FFN weights prefetch during the
attention tail.
"""

import sys

try:
    import concourse  # noqa: F401
except ImportError:
    sys.path.insert(0, "/opt/trn_rl_repo")

import numpy as np
import ml_dtypes

import concourse.bass as bass
import concourse.bacc as bacc
import concourse.tile as tile
from concourse import mybir
from concourse.bass_utils import run_bass_kernel_spmd

F32 = mybir.dt.float32
BF16 = mybir.dt.bfloat16
AF = mybir.ActivationFunctionType
ALU = mybir.AluOpType
PSUM = bass.MemorySpace.PSUM

B, T, D = 4, 2048, 1024
H, HD = 16, 64
HID = 4 * D
EPS = 1e-5
N_CORES = 8
NB = T // 128  # 16 token blocks per batch
NQ = 8  # query slots per core
TQ = NQ * 128  # query tokens per core

QB = [
    [15, 12, 11, 8, 7, 4, 3, 0],
    [14, 13, 10, 9, 6, 5, 2, 1],
]
NKMAX = [16, 14, 12, 10, 8, 6, 4, 2]


def _perm(p):
    idx = []
    for qb in QB[p]:
        idx.extend(range(qb * 128, (qb + 1) * 128))
    return np.array(idx, dtype=np.int64)


def _masks(p):
    """[NQ, 128, 256] multiplicative mask for the last two key blocks per slot."""
    m = np.zeros((NQ, 128, 256), dtype=np.float32)
    tri = np.triu(np.ones((128, 128), np.float32))  # valid where k_local <= q_local
    for j in range(NQ):
        nk = QB[p][j] + 1
        nkm = NKMAX[j]
        for c in range(2):
            ki = nkm - 2 + c
            if ki < nk - 1:
                m[j, :, c * 128:(c + 1) * 128] = 1.0
            elif ki == nk - 1:
                m[j, :, c * 128:(c + 1) * 128] = tri
    return m


# ---------------------------------------------------------------------------
# Device program
# ---------------------------------------------------------------------------

def _ln_stats(nc, pool, x_ap, eps_t):
    """LN stats of a [128, D] fp32 tile -> (rstd, nmr) [128,1] tiles."""
    stats = pool.tile([128, 2, nc.vector.BN_STATS_DIM], F32, tag="ln_stats")
    mv = pool.tile([128, nc.vector.BN_AGGR_DIM], F32, tag="ln_mv")
    nmr = pool.tile([128, 1], F32, tag="ln_nmr")
    xg = x_ap.rearrange("p (s d) -> p s d", s=2)
    for s in range(2):
        nc.vector.bn_stats(out=stats[:, s, :], in_=xg[:, s, :])
    nc.vector.bn_aggr(out=mv[:], in_=stats[:])
    mean = mv[:, 0:1]
    rstd = mv[:, 1:2]
    nc.scalar.activation(out=rstd, in_=rstd, func=AF.Sqrt, bias=eps_t[:], scale=1.0)
    nc.vector.reciprocal(out=rstd, in_=rstd)
    nc.vector.tensor_scalar(
        out=nmr[:], in0=mean, scalar1=rstd, scalar2=-1.0,
        op0=ALU.mult, op1=ALU.mult,
    )
    return rstd, nmr


def build_program(niter=None, stop_after=None):
    nc = bacc.Bacc("TRN2", target_bir_lowering=False, debug=False,
                   num_devices=N_CORES)

    xf_d = nc.dram_tensor("x_full", [T, D], F32, kind="ExternalInput")
    xq_d = nc.dram_tensor("x_q", [TQ, D], F32, kind="ExternalInput")
    mask_d = nc.dram_tensor("mask", [128, NQ, 256], BF16, kind="ExternalInput")
    wqkv_d = nc.dram_tensor("wqkv", [128, 8, 3 * D], BF16, kind="ExternalInput")
    wproj_d = nc.dram_tensor("wproj", [128, 8, D], BF16, kind="ExternalInput")
    wff1_d = nc.dram_tensor("wff1", [128, 8, HID], BF16, kind="ExternalInput")
    wff2_d = nc.dram_tensor("wff2", [128, 32, D], BF16, kind="ExternalInput")
    bqk_d = nc.dram_tensor("bqk", [128, 16], F32, kind="ExternalInput")
    bv_d = nc.dram_tensor("bv", [D], F32, kind="ExternalInput")
    bproj_d = nc.dram_tensor("bproj", [D], F32, kind="ExternalInput")
    bff1_d = nc.dram_tensor("bff1", [128, 32], F32, kind="ExternalInput")
    bff2_d = nc.dram_tensor("bff2", [D], F32, kind="ExternalInput")
    g1_d = nc.dram_tensor("g1", [128, 8], F32, kind="ExternalInput")
    b1_d = nc.dram_tensor("b1", [128, 8], F32, kind="ExternalInput")
    g2_d = nc.dram_tensor("g2", [128, 8], F32, kind="ExternalInput")
    b2_d = nc.dram_tensor("b2", [128, 8], F32, kind="ExternalInput")
    y_d = nc.dram_tensor("y", [TQ, D], F32, kind="ExternalOutput")

    def bcast_row(dram_handle, n):
        ap = dram_handle.ap()
        return bass.AP(tensor=ap.tensor, offset=ap.offset, ap=[[0, 128], [1, n]])

    dram = dict(xf=xf_d, xq=xq_d, wqkv=wqkv_d, wproj=wproj_d,
                wff1=wff1_d, wff2=wff2_d, y=y_d)

    with tile.TileContext(nc) as tc:
        with tc.tile_pool(name="singles", bufs=1) as singles:
            ident = singles.tile([128, 128], BF16)
            from concourse.masks import make_identity
            make_identity(nc, ident[:])
            nc._ident = ident

            st = {}
            st["eps"] = singles.tile([128, 1], F32, name="eps", tag="eps")
            nc.vector.memset(st["eps"][:], EPS)
            for nm, dt, shape, src in (
                ("g1", F32, [128, 8], g1_d), ("b1", F32, [128, 8], b1_d),
                ("g2", F32, [128, 8], g2_d), ("b2", F32, [128, 8], b2_d),
                ("bqk", F32, [128, 16], bqk_d), ("bff1", F32, [128, 32], bff1_d),
                ("mask", BF16, [128, NQ, 256], mask_d),
            ):
                st[nm] = singles.tile(shape, dt, name=nm, tag=nm)
                nc.sync.dma_start(st[nm][:], src.ap())
            for nm, src in (("vb", bv_d), ("pb", bproj_d), ("fb2", bff2_d)):
                st[nm] = singles.tile([128, D], F32, name=nm, tag=nm)
                nc.gpsimd.dma_start(st[nm][:], bcast_row(src, D))

            if niter is None:
                _block_body(tc, nc, dram, st, stop_after)
            else:
                with tc.For_i(0, niter, 1):
                    _block_body(tc, nc, dram, st, stop_after)

    nc.compile()
    return nc


def _block_body(tc, nc, dram, st, stop_after=None):
    xf_d, xq_d, y_d = dram["xf"], dram["xq"], dram["y"]
    eps_t = st["eps"]
    x2_d = nc.dram_tensor("x2_scratch", [TQ, D], F32, kind="Internal")

    # --- persistent SBUF (right side, LIFO release order) -----------------
    # XTMP serves as XTQ during the prefix and is recycled as XT2 during
    # attention (XTQ's last read precedes XT2's first write; tile WAR deps
    # order them).  w1p sits at the bottom of the left stack so it can
    # outlive the attention pools into ff1.
    w1p = tc.alloc_tile_pool(name="w1", bufs=4)
    xtmp_p = tc.alloc_tile_pool(name="xtmp", bufs=1, side="right")
    XTQ = xtmp_p.tile([128, 8, TQ], BF16, name="XTMP", tag="xtmp")
    XT2 = XTQ
    qkv_p = tc.alloc_tile_pool(name="qkv", bufs=1, side="right")
    QT = qkv_p.tile([128, 8, TQ], BF16, name="QT", tag="qt")
    KT = qkv_p.tile([128, 8, T], BF16, name="KT", tag="kt")
    V = qkv_p.tile([128, NB, H, HD + 1], BF16, name="V", tag="v")
    for t in range(NB):
        nc.vector.memset(V[:, t, :, HD:HD + 1], 1.0)

    xtf_p = tc.alloc_tile_pool(name="xtf", bufs=1, side="right")
    XTF = xtf_p.tile([128, 8, T], BF16, name="XTF", tag="xtf")

    # --- PSUM pools: sc 3x[128,1024]f32 (6 banks) + u 2x2KB (2 banks) -----
    sc_ps = tc.alloc_tile_pool(name="sc_ps", bufs=2, space=PSUM)
    u_ps = tc.alloc_tile_pool(name="u_ps", bufs=4, space=PSUM)

    def sc_tile():
        return sc_ps.tile([128, 1024], F32, tag="sc", name="ps")

    def u_tile(dt=F32):
        if dt == F32:
            return u_ps.tile([128, 512], dt, tag="u", name="ups")
        return u_ps.tile([128, 512], dt, tag="u", name="ups",
                         padded_shape=[128, 1024])

    # --- prefix pools (right side, above xtf) -----------------------------
    wqk_p = tc.alloc_tile_pool(name="wqk", bufs=3, side="right")
    wv_p = tc.alloc_tile_pool(name="wv", bufs=2, side="right")
    p1 = tc.alloc_tile_pool(name="p1", bufs=5, side="right")
    p1s = tc.alloc_tile_pool(name="p1s", bufs=8, side="right")
    xln_p = tc.alloc_tile_pool(name="xln", bufs=4, side="right")

    # ---- LN1 pipeline pieces --------------------------------------------
    def ln1_tile(x_d, row0, chunks=1):
        """Load [128, D] from x_d at row0, LN -> bf16 tile (no gain yet).

        chunks>1 splits the load across DMA queues to cut arrival latency
        (matters for the first tiles after the For_i iteration barrier)."""
        xt = p1.tile([128, D], F32, tag="xt", name="xt")
        cw = D // chunks
        for c in range(chunks):
            nc.sync.dma_start(xt[:, c * cw:(c + 1) * cw],
                              x_d.ap()[row0:row0 + 128, c * cw:(c + 1) * cw])
        ln = xln_p.tile([128, D], BF16, tag="ln", name="ln")
        rstd, nmr = _ln_stats(nc, p1s, xt[:], eps_t)
        nc.scalar.activation(out=ln[:], in_=xt[:], func=AF.Identity,
                             bias=nmr[:], scale=rstd)
        return ln

    def tp_group(tiles4, dst3, col0):
        """Transpose 4 LN tiles' dim-chunks into dst3[:, dc, col0:col0+512].

        g1/b1 fused into the psum->sbuf copy (DVE even dc / ACT odd dc)."""
        for dc in range(8):
            pt = u_tile(BF16)
            for u in range(4):
                nc.tensor.transpose(
                    pt[:, u * 128:(u + 1) * 128],
                    tiles4[u][:, dc * 128:(dc + 1) * 128], nc._ident[:])
            dst = dst3[:, dc, col0:col0 + 512]
            if dc % 2 == 1:
                nc.scalar.activation(
                    out=dst, in_=pt[:], func=AF.Identity,
                    bias=st["b1"][:, dc:dc + 1], scale=st["g1"][:, dc:dc + 1])
            else:
                nc.vector.tensor_scalar(
                    out=dst, in0=pt[:],
                    scalar1=st["g1"][:, dc:dc + 1], scalar2=st["b1"][:, dc:dc + 1],
                    op0=ALU.mult, op1=ALU.add)

    def make_qk_half(cc, pair, h2, w):
        """One [128,512] half of a QT (cc<8) / KT (cc>=8) chunk via u_ps."""
        is_q = cc < 8
        XT = XTQ if is_q else XTF
        OUT = QT if is_q else KT
        occ = cc if is_q else cc - 8
        ps = u_tile()
        for dc in range(8):
            nc.tensor.matmul(
                ps[:], w[:, dc, :],
                XT[:, dc, (pair * 2 + h2) * 512:(pair * 2 + h2 + 1) * 512],
                start=(dc == 0), stop=(dc == 7),
            )
        nc.vector.tensor_scalar_add(
            out=OUT[:, occ, (pair * 2 + h2) * 512:(pair * 2 + h2 + 1) * 512],
            in0=ps[:], scalar1=st["bqk"][:, cc:cc + 1],
        )

    def load_wqk(cc):
        w = wqk_p.tile([128, 8, 128], BF16, tag="wqk", name="w")
        nc.sync.dma_start(
            w[:], dram["wqkv"].ap()[:, :, cc * 128:(cc + 1) * 128])
        return w

    def v_block(t, vh, wv):
        """V for token block t, head-half vh (8 heads x 64) via u_ps."""
        ps = u_tile()
        for dc in range(8):
            nc.tensor.matmul(
                ps[:], XTF[:, dc, t * 128:(t + 1) * 128],
                wv[:, dc, :], start=(dc == 0), stop=(dc == 7),
            )
        nc.vector.tensor_add(
            out=V[:, t, vh * 8:(vh + 1) * 8, 0:HD],
            in0=ps[:].rearrange("p (h d) -> p h d", h=8),
            in1=st["vb"][:, vh * 512:(vh + 1) * 512]
                .rearrange("p (h d) -> p h d", h=8),
        )

    # =====================================================================
    # PREFIX: pipelined LN1 -> transposes -> K0 / Q / V.  Emission order is
    # the PE schedule; LN for later groups overlaps earlier PE matmuls.
    # =====================================================================
    wv0 = wv_p.tile([128, 8, 512], BF16, tag="wv", name="wv0")
    nc.sync.dma_start(wv0[:], dram["wqkv"].ap()[:, :, 2 * D:2 * D + 512])
    wv1 = wv_p.tile([128, 8, 512], BF16, tag="wv", name="wv1")
    nc.sync.dma_start(wv1[:], dram["wqkv"].ap()[:, :, 2 * D + 512:3 * D])

    # xf group 0 (blocks 0..3) -> XTF cols 0..512 (chunked loads: these
    # gate the whole pipeline right after the iteration barrier)
    tiles = [ln1_tile(xf_d, u * 128, chunks=4) for u in range(4)]
    tp_group(tiles, XTF, 0)

    # K pair-0 lower halves (tokens 0..512) on PE; xf group 1 + xq LN
    # tiles stream on ACT/DVE meanwhile.  Weight chunks are re-loaded per
    # half-pass (cheap: gpsimd-issued DMA) to keep the wqk pool at 3 bufs.
    ln_feed = [lambda u=u: ln1_tile(xf_d, (4 + u) * 128) for u in range(4)]
    ln_feed += [lambda u=u: ln1_tile(xq_d, u * 128) for u in range(8)]
    ln_out = []

    def feed(n):
        for _ in range(n):
            if ln_feed:
                ln_out.append(ln_feed.pop(0)())

    wk = [load_wqk(8), load_wqk(9)]
    for i, cc in enumerate(range(8, 16)):
        if cc + 2 < 16:
            wk.append(load_wqk(cc + 2))
        feed(1)
        make_qk_half(cc, 0, 0, wk[i])
    tp_group(ln_out[0:4], XTF, 512)  # xf group 1

    # K pair-0 upper halves (tokens 512..1024); remaining xq LN streams
    wk = [load_wqk(8), load_wqk(9)]
    for i, cc in enumerate(range(8, 16)):
        if cc + 2 < 16:
            wk.append(load_wqk(cc + 2))
        feed(1)
        make_qk_half(cc, 0, 1, wk[i])
    tp_group(ln_out[4:8], XTQ, 0)
    tp_group(ln_out[8:12], XTQ, 512)

    # Q (all chunks, both halves); xf group 2 LN meanwhile
    ln_feed = [lambda u=u: ln1_tile(xf_d, (8 + u) * 128) for u in range(8)]
    ln_out = []
    wk = [load_wqk(0), load_wqk(1)]
    for cc in range(8):
        if cc + 2 < 8:
            wk.append(load_wqk(cc + 2))
        feed(1)
        make_qk_half(cc, 0, 0, wk[cc])
        make_qk_half(cc, 0, 1, wk[cc])

    # V blocks 0..7; xf group 3 LN meanwhile
    for t in range(8):
        feed(1)
        v_block(t, 0, wv0)
        v_block(t, 1, wv1)
    # xf groups 2,3 transposes -> XTF cols 1024..2048
    tp_group(ln_out[0:4], XTF, 1024)
    tp_group(ln_out[4:8], XTF, 1536)
    # V blocks 8..15
    for t in range(8, 16):
        v_block(t, 0, wv0)
        v_block(t, 1, wv1)

    xln_p.release()
    p1s.release()
    p1.release()
    wv_p.release()
    if stop_after == "prefix":
        wqk_p.release()
        xtf_p.release()
        qkv_p.release()
        xtmp_p.release()
        u_ps.release()
        sc_ps.release()
        w1p.release()
        return

    # =====================================================================
    # ATTENTION: stages = (slot j desc, head h asc). Injected between
    # stages: K pair 1 halves, per-slot proj/LN2/XT2, weight prefetch.
    # =====================================================================
    atp = tc.alloc_tile_pool(name="at", bufs=4)
    epp = tc.alloc_tile_pool(name="ep", bufs=4)
    attt_pool = tc.alloc_tile_pool(name="atttsl", bufs=2)
    att_pool = tc.alloc_tile_pool(name="attsl", bufs=2)
    x2t_pool = tc.alloc_tile_pool(name="x2tp", bufs=1)
    xqp = tc.alloc_tile_pool(name="xqp", bufs=1)
    ln2p = tc.alloc_tile_pool(name="ln2p", bufs=2)
    wpp = tc.alloc_tile_pool(name="wproj", bufs=1)

    PW = wpp.tile([128, 8, D], BF16, name="PW", tag="pw")
    nc.sync.dma_start(PW[:], dram["wproj"].ap())

    # ---- injected tasks (popped 1-2 per stage, FIFO) --------------------
    inj = []
    k1_w = {}

    def mk_k1_load(cc):
        def run():
            k1_w[cc] = load_wqk(cc)
        return run

    inj.append(mk_k1_load(8))
    for cc in range(8, 16):
        if cc + 1 < 16:
            inj.append(mk_k1_load(cc + 1))
        inj.append(lambda cc=cc: make_qk_half(cc, 1, 0, k1_w[cc]))
        inj.append(lambda cc=cc: make_qk_half(cc, 1, 1, k1_w[cc]))
    n_k1_tasks = len(inj)

    # slot tail pieces; jst = [j, att_j, xq, x2t] then += [attt, ln2]
    def tail_attt(jst):
        j, att_j = jst[0], jst[1]
        attt = attt_pool.tile([128, 8, 128], BF16, tag="attt", name="attt")
        jst.append(attt)
        for g2 in range(2):
            pt = u_tile(BF16)
            for u in range(4):
                dc = g2 * 4 + u
                nc.tensor.transpose(
                    pt[:, u * 128:(u + 1) * 128],
                    att_j[:, dc * 128:(dc + 1) * 128], nc._ident[:])
            nc.vector.tensor_copy(attt[:, g2 * 4:(g2 + 1) * 4, :], pt[:])

    def tail_proj_half(jst, half):
        j, att_j, xq, x2t, attt = jst[0], jst[1], jst[2], jst[3], jst[4]
        ps = u_tile()
        for hcc in range(8):
            nc.tensor.matmul(
                ps[:], attt[:, hcc, :],
                PW[:, hcc, half * 512:(half + 1) * 512],
                start=(hcc == 0), stop=(hcc == 7),
            )
        sl = slice(half * 512, (half + 1) * 512)
        nc.vector.tensor_add(out=x2t[:, sl], in0=ps[:], in1=st["pb"][:, sl])
        nc.vector.tensor_add(out=x2t[:, sl], in0=x2t[:, sl], in1=xq[:, sl])

    def tail_x2out(jst):
        j, x2t = jst[0], jst[3]
        nc.sync.dma_start(x2_d.ap()[j * 128:(j + 1) * 128, :], x2t[:])
        ln2 = ln2p.tile([128, D], BF16, tag="ln2", name="ln2")
        rstd, nmr = _ln_stats(nc, epp, x2t[:], eps_t)
        nc.vector.tensor_scalar(
            out=ln2[:], in0=x2t[:], scalar1=rstd, scalar2=nmr[:],
            op0=ALU.mult, op1=ALU.add)
        jst.append(ln2)

    def tail_xt2(jst, g2):
        j, ln2 = jst[0], jst[5]
        pt = u_tile(BF16)
        for u in range(4):
            dc = g2 * 4 + u
            nc.tensor.transpose(
                pt[:, u * 128:(u + 1) * 128],
                ln2[:, dc * 128:(dc + 1) * 128], nc._ident[:])
        for u in range(4):
            dc = g2 * 4 + u
            nc.vector.tensor_scalar(
                out=XT2[:, dc, j * 128:(j + 1) * 128],
                in0=pt[:, u * 128:(u + 1) * 128],
                scalar1=st["g2"][:, dc:dc + 1], scalar2=st["b2"][:, dc:dc + 1],
                op0=ALU.mult, op1=ALU.add)

    # ---- ff1 upper-half (XT2 cols 512:1024 = slots 4..7) interleaved into
    # late attention as PE filler.  Pre-gelu bf16 staged in P1; the gelu
    # (ACT) is deferred to the ff phase where ACT is idle, so the
    # exp-saturated attention window gains no ACT work.
    w1h_tiles = []
    p1_tiles = []
    late = {}

    def mk_w1h_load(hh):
        def run():
            w1 = w1p.tile([128, 8, 128], BF16, tag="w1")
            nc.sync.dma_start(
                w1[:], dram["wff1"].ap()[:, :, hh * 128:(hh + 1) * 128])
            w1h_tiles.append(w1)
        return run

    def mk_ff1h1_mm(hh):
        def run():
            ps = u_tile()
            for dc in range(8):
                nc.tensor.matmul(
                    ps[:], w1h_tiles[hh][:, dc, :],
                    XT2[:, dc, 512:1024],
                    start=(dc == 0), stop=(dc == 7),
                )
            p1t = late["p1h"].tile([128, 512], BF16, tag="p1h", name="p1t")
            p1_tiles.append(p1t)
            nc.vector.tensor_copy(p1t[:], ps[:])
        return run

    ff1q = []
    ff1q.append(mk_w1h_load(0))
    ff1q.append(mk_w1h_load(1))
    for hh in range(32):
        if hh + 2 < 32:
            ff1q.append(mk_w1h_load(hh + 2))
        ff1q.append(mk_ff1h1_mm(hh))

    # ---- attention stages ------------------------------------------------
    def emit_scores(j, h, att_j):
        hc, po = h >> 1, 64 * (h & 1)
        nblk = NKMAX[j]
        groups = []
        for g in range((nblk + 7) // 8):
            blo = g * 8
            bhi = min(blo + 8, nblk)
            ncol = (bhi - blo) * 128
            ps = sc_tile()
            for kk in range(blo, bhi):
                nc.tensor.matmul(
                    ps[:, (kk - blo) * 128:(kk - blo + 1) * 128],
                    KT[po:po + 64, hc, kk * 128:(kk + 1) * 128],
                    QT[po:po + 64, hc, j * 128:(j + 1) * 128],
                    start=True, stop=True,
                )
            at = atp.tile([128, 1024], BF16, tag="at", name="at")
            nc.scalar.activation(out=at[:, 0:ncol], in_=ps[:, 0:ncol],
                                 func=AF.Exp, scale=0.125)
            for kk in range(max(blo, nblk - 2), bhi):
                mc = (kk - (nblk - 2)) * 128
                nc.vector.tensor_mul(
                    out=at[:, (kk - blo) * 128:(kk - blo + 1) * 128],
                    in0=at[:, (kk - blo) * 128:(kk - blo + 1) * 128],
                    in1=st["mask"][:, j, mc:mc + 128],
                )
            groups.append((blo, bhi, at))
        return groups

    def emit_pv(pend):
        h, j, groups, att_j = pend
        nblk = NKMAX[j]
        pv = u_tile()
        for blo, bhi, at in groups:
            for kk in range(blo, bhi):
                nc.tensor.matmul(
                    pv[:, 0:HD + 1],
                    at[:, (kk - blo) * 128:(kk - blo + 1) * 128],
                    V[:, kk, h, :],
                    start=(kk == 0), stop=(kk == nblk - 1),
                )
        r = epp.tile([128, 1], F32, tag="recip", name="r")
        nc.vector.reciprocal(out=r[:], in_=pv[:, HD:HD + 1])
        nc.vector.tensor_scalar_mul(
            out=att_j[:, h * HD:(h + 1) * HD],
            in0=pv[:, 0:HD], scalar1=r[:],
        )

    deferred = []

    def queue_tail(jst):
        # xt2 transposes of the PREVIOUS slot are queued behind this slot's
        # tail so the PE doesn't catch up with the DVE ln2 chain
        inj.append(lambda: tail_attt(jst))
        inj.append(lambda: tail_proj_half(jst, 0))
        inj.append(lambda: tail_proj_half(jst, 1))
        inj.append(lambda: tail_x2out(jst))
        inj.extend(deferred)
        deferred.clear()
        deferred.append(lambda: tail_xt2(jst, 0))
        deferred.append(lambda: tail_xt2(jst, 1))

    pending = None
    xtf_released = False
    k1_left = n_k1_tasks
    stage_i = 0
    for j in reversed(range(NQ)):
        att_j = att_pool.tile([128, D], BF16, tag="att", name="att_j")
        # prefetch this slot's xq + allocate its x2t up front
        xq = xqp.tile([128, D], F32, tag="xq", name="xq")
        nc.sync.dma_start(xq[:], xq_d.ap()[j * 128:(j + 1) * 128, :])
        x2t = x2t_pool.tile([128, D], F32, tag="x2t", name="x2t")
        jst = [j, att_j, xq, x2t]
        for h in range(H):
            groups = emit_scores(j, h, att_j)
            if pending is not None:
                emit_pv(pending)
            pending = (h, j, groups, att_j)
            stage_i += 1
            # drain K1 aggressively (3/stage) so XTF can release early;
            # afterwards 1-2 tasks per stage, tails first, ff1h1 as filler
            npop = 3 if k1_left > 0 else (
                2 if len(inj) > 8 and stage_i % 2 == 0 else 1)
            for _ in range(min(npop, len(inj))):
                inj.pop(0)()
                k1_left -= 1
            if k1_left <= 0 and not xtf_released:
                wqk_p.release()
                xtf_p.release()
                xtf_released = True
            if stage_i == 84:
                late["p1h"] = tc.alloc_tile_pool(name="p1h", bufs=32)
            if stage_i >= 90 and ff1q:
                if not inj:
                    ff1q.pop(0)()
                if ff1q and stage_i % 2 == 0:
                    ff1q.pop(0)()
        queue_tail(jst)
    emit_pv(pending)
    while inj:
        inj.pop(0)()
    while ff1q:
        ff1q.pop(0)()
    for f in deferred:
        f()

    qkv_p.release()

    # =====================================================================
    # FFN.  HT upper half comes from the staged pre-gelu P1 tiles (gelu on
    # the now-idle ACT engine); lower half is computed here.
    # =====================================================================
    ht_p = tc.alloc_tile_pool(name="ht", bufs=1, side="right")
    HT = ht_p.tile([128, 32, TQ], BF16, name="HT", tag="ht")
    for hh in range(32):
        nc.scalar.activation(
            out=HT[:, hh, 512:1024], in_=p1_tiles[hh][:], func=AF.Gelu,
            bias=st["bff1"][:, hh:hh + 1], scale=1.0)
    late["p1h"].release()
    wpp.release()
    ln2p.release()
    xqp.release()
    x2t_pool.release()
    att_pool.release()
    attt_pool.release()
    epp.release()
    atp.release()

    w1_tiles0 = []

    def load_w1_h0(hh):
        w1 = w1p.tile([128, 8, 128], BF16, tag="w1")
        nc.sync.dma_start(
            w1[:], dram["wff1"].ap()[:, :, hh * 128:(hh + 1) * 128])
        w1_tiles0.append(w1)

    load_w1_h0(0)
    load_w1_h0(1)
    for hh in range(32):
        if hh + 2 < 32:
            load_w1_h0(hh + 2)
        ps = u_tile()
        for dc in range(8):
            nc.tensor.matmul(
                ps[:], w1_tiles0[hh][:, dc, :],
                XT2[:, dc, 0:512],
                start=(dc == 0), stop=(dc == 7),
            )
        nc.scalar.activation(
            out=HT[:, hh, 0:512], in_=ps[:], func=AF.Gelu,
            bias=st["bff1"][:, hh:hh + 1], scale=1.0)

    x2f_p = tc.alloc_tile_pool(name="x2f", bufs=1, side="right")
    X2F = x2f_p.tile([128, NQ, D], F32, name="X2F", tag="x2f")
    with tc.tile_pool(name="x2f_ld", bufs=3) as x2fp:
        for t in range(NQ):
            x2l = x2fp.tile([128, D], F32, tag="x2l", name="x2l")
            nc.sync.dma_start(x2l[:], x2_d.ap()[t * 128:(t + 1) * 128, :])
            nc.vector.tensor_add(out=X2F[:, t, :], in0=x2l[:], in1=st["fb2"][:])
    u_ps.release()
    sc_ps.release()

    with tc.tile_pool(name="w2", bufs=6) as w2p, \
         tc.tile_pool(name="yp", bufs=3) as yp, \
         tc.tile_pool(name="ff2ps", bufs=4, space=PSUM) as ff2ps:
        for tg in range(2):
            pss = [ff2ps.tile([128, 1024], F32, name="ym", tag="ym")
                   for _ in range(4)]
            for hh in range(32):
                w2 = w2p.tile([128, D], BF16, tag="w2")
                nc.sync.dma_start(w2[:], dram["wff2"].ap()[:, hh, :])
                for tt in range(4):
                    t = tg * 4 + tt
                    for half in range(2):
                        nc.tensor.matmul(
                            pss[tt][:, half * 512:(half + 1) * 512],
                            HT[:, hh, t * 128:(t + 1) * 128],
                            w2[:, half * 512:(half + 1) * 512],
                            start=(hh == 0), stop=(hh == 31),
                        )
            for tt in range(4):
                t = tg * 4 + tt
                yt = yp.tile([128, D], F32, tag="yt")
                nc.vector.tensor_add(out=yt[:], in0=pss[tt][:], in1=X2F[:, t, :])
                nc.sync.dma_start(
                    y_d.ap()[t * 128:(t + 1) * 128, :], yt[:])
    x2f_p.release()
    ht_p.release()
    xtmp_p.release()
    w1p.release()


# ---------------------------------------------------------------------------
# Host wrapper
# ---------------------------------------------------------------------------

_PROG_CACHE = {}


def _get_program(niter=None):
    if niter not in _PROG_CACHE:
        _PROG_CACHE[niter] = build_program(niter)
    return _PROG_CACHE[niter]


def make_in_maps(x, ln1_g, ln1_b, qkv_w, qkv_b, proj_w, proj_b,
                 ln2_g, ln2_b, ff1_w, ff1_b, ff2_w, ff2_b):
    bf = ml_dtypes.bfloat16
    f32 = np.float32

    def pcol(v, n):  # [n*128] -> [128, n] (chunk c holds elements c*128..c*128+127)
        return np.ascontiguousarray(np.asarray(v, f32).reshape(n, 128).T)

    def dimmajor(w, nchunk, ncol):  # [nchunk*128, ncol] -> [128, nchunk, ncol]
        return np.ascontiguousarray(
            np.asarray(w, f32).reshape(nchunk, 128, ncol).transpose(1, 0, 2)
        ).astype(bf)

    qkv_b = np.asarray(qkv_b, f32)
    common = dict(
        wqkv=dimmajor(qkv_w, 8, 3 * D),
        wproj=dimmajor(proj_w, 8, D),
        wff1=dimmajor(ff1_w, 8, HID),
        wff2=dimmajor(ff2_w, 32, D),
        bqk=np.ascontiguousarray(
            np.concatenate([pcol(qkv_b[0:D], 8), pcol(qkv_b[D:2 * D], 8)], axis=1)),
        bv=qkv_b[2 * D:3 * D].copy(),
        bproj=np.asarray(proj_b, f32).copy(),
        bff1=pcol(ff1_b, 32),
        bff2=np.asarray(ff2_b, f32).copy(),
        g1=pcol(ln1_g, 8), b1=pcol(ln1_b, 8),
        g2=pcol(ln2_g, 8), b2=pcol(ln2_b, 8),
    )
    masks = [np.ascontiguousarray(_masks(p).transpose(1, 0, 2)).astype(bf)
             for p in range(2)]
    perms = [_perm(0), _perm(1)]

    x = np.asarray(x, f32)
    in_maps = []
    for c in range(N_CORES):
        b, p = c // 2, c % 2
        m = dict(common)
        m["x_full"] = np.ascontiguousarray(x[b])
        m["x_q"] = np.ascontiguousarray(x[b][perms[p]])
        m["mask"] = masks[p]
        in_maps.append(m)
    return in_maps, perms


def kernel(**inputs):
    in_maps, perms = make_in_maps(**{k: np.asarray(v) for k, v in inputs.items()})
    nc = _get_program()
    res = run_bass_kernel_spmd(nc, in_maps, list(range(N_CORES))).results
    y = np.empty((B, T, D), np.float32)
    for c in range(N_CORES):
        b, p = c // 2, c % 2
        y[b][perms[p]] = res[c]["y"]
    return y
